# revision 75
# baseline (speedup 1.0000x reference)
"""DeepseekV3 decoder layer on 8 Trainium2 NeuronCores (Bass/Tile).

Sharding: tensor-parallel heads for MLA (2 heads/core), token shards
(256 tok/core) for projections/norms/router/shared-expert, expert-
parallel MoE (1 routed expert/core over all tokens, weights resident).

Collectives (all bf16, each overlapped with independent compute):
AllGather(kv latents) under q_a compute -> AllGather(q latents) under
the attention kv path -> AllToAll(attn out) -> AllGather(h) under
router + shared expert -> AllGather(routed weights) -> 4x chunked
ReduceScatter(expert partials) pipelined against the down-projection.
Latents are pre-normalized (q_a/kv_a rmsnorm folded in pre-gather), so
no rstd rows travel. Collective-gated loads ride the Activation HWDGE
ring / SWDGE so they never head-block the weight-streaming SP ring;
SBUF-side DMA access patterns always keep the partition dim outermost.

Activations kept feature-major [feat_p, tok_f]; weights pre-transposed on
host to [K, M] so each matmul is lhsT[K,M].T @ rhs[K,N], K = contraction.
Matmuls run bf16 except scores/attention (fp32r) and router (fp32).
"""
import sys

if "/opt/trn_rl_repo" not in sys.path:
    sys.path.insert(0, "/opt/trn_rl_repo")

import numpy as np
import ml_dtypes

import concourse.bass as bass
import concourse.bacc as bacc
import concourse.tile as tile
from concourse import mybir
from concourse import bass_utils
from concourse.tile import add_dep_helper

FP = mybir.dt.float32
BF = mybir.dt.bfloat16
FR = mybir.dt.float32r
AF = mybir.ActivationFunctionType
ALU = mybir.AluOpType

NCORE = 8
B, S, H = 2, 1024, 2048
T = B * S
NH, DN, DR, DV = 16, 128, 64, 128
DQK = DN + DR
KVL, QL = 512, 1536
NE, NG, INTER = 8, 4, 768
TSH = T // NCORE          # 256
HPC = NH // NCORE         # 2
SCALING = float(DQK) ** -0.5
RSF = 2.5
EPS = 1e-6

AG1_ROWS = QL + KVL + DR   # 2112 (latents pre-normalized, bf16)


def fr(ap):
    return ap.bitcast(FR)


def build_program():
    nc = bacc.Bacc("TRN2", target_bir_lowering=False, debug=False,
                   num_devices=NCORE)

    def din(name, shape, dtype=FP):
        return nc.dram_tensor(name, shape, dtype, kind="ExternalInput").ap()

    hidT = din("hidT", [H, TSH])
    qa_wT = din("qa_wT", [H, QL], BF)
    kva_wT = din("kva_wT", [H, KVL + DR], BF)
    qb_wT = din("qb_wT", [QL, HPC * DQK], BF)  # cols: h0n h1n h0A h0B h1A h1B
    kvb_wT = din("kvb_wT", [KVL, HPC * 256], BF)  # cols: k0 k1 v0 v1
    o_wT = din("o_wT", [NH * DV, H], BF)
    r_wT = din("r_wT", [H, NE])
    r_bias = din("r_bias", [NE, 1])
    onehot = din("onehot", [NE, 1], BF)
    g_wT = din("g_wT", [H, INTER], BF)         # this core's expert
    u_wT = din("u_wT", [H, INTER], BF)
    d_wT = din("d_wT", [INTER, H], BF)
    sg_wT = din("sg_wT", [H, INTER], BF)
    su_wT = din("su_wT", [H, INTER], BF)
    sd_wT = din("sd_wT", [INTER, H], BF)
    cc_q = din("cc_q", [128, T])
    ss_q = din("ss_q", [128, T])
    cc_k = din("cc_k", [DR, TSH])
    ss_k = din("ss_k", [DR, TSH])
    maskT_d = din("maskT", [512, 512], BF)
    Gm_d = din("Gm", [NE, NG])
    Dg_d = din("Dg", [NG, NG * NG])
    Rg_d = din("Rg", [NG * NG, NG])
    Em_d = din("Em", [NG, NE])
    De_d = din("De", [NE, NE * NE])
    Re_d = din("Re", [NE * NE, NE])

    out = nc.dram_tensor("out", [H, TSH], FP, kind="ExternalOutput").ap()

    RG = [list(range(NCORE))]

    def dma(out_ap, in_ap):
        nc.sync.dma_start(out_ap, in_ap)

    def kp(ap, p=128):
        return ap.rearrange("(k p) t -> p k t", p=p)

    tcx = tile.TileContext(nc)
    tc = tcx.__enter__()
    dram_cm = tc.tile_pool(name="dram", bufs=1, space="DRAM")
    dram = dram_cm.__enter__()
    pp_cm = tc.tile_pool(name="persist", bufs=1)
    pp = pp_cm.__enter__()

    # latent gather split in two so each AllGather hides under phase-A
    # compute of the other half
    ag1a_in = dram.tile([QL, TSH], BF)
    ag1a_out = dram.tile([NCORE * QL, TSH], BF, addr_space="Shared")
    ag1b_in = dram.tile([KVL + DR, TSH], BF)
    ag1b_out = dram.tile([NCORE * (KVL + DR), TSH], BF, addr_space="Shared")
    a2a_in = dram.tile([NCORE * 256, TSH], BF)
    a2a_out = dram.tile([NCORE * 256, TSH], BF)
    ag2_in = dram.tile([H, TSH], BF)
    ag2_out = dram.tile([NCORE * H, TSH], BF, addr_space="Shared")
    agw_in = dram.tile([NE, TSH], BF)
    agw_out = dram.tile([NCORE * NE, TSH], BF, addr_space="Shared")
    # expert partials in 4 independent m-chunks (separate tiles so chunk
    # c+1's writes carry no false dependency on chunk c's ReduceScatter);
    # chunk rows = j*512 + mi*128 + p  (j home core, mi = m%4)
    rs_ins = [dram.tile([NCORE * 512, TSH], BF, name=f"rs_in{i}")
              for i in range(4)]
    rs_outs = [dram.tile([512, TSH], BF, name=f"rs_out{i}")
               for i in range(4)]

    ones = pp.tile([128, 1], FP)
    nc.vector.memset(ones[:], 1.0)
    epsb = pp.tile([128, 1], FP)
    nc.vector.memset(epsb[:], EPS)

    ag1av = ag1a_out.rearrange("(j r) t -> j r t", r=QL)
    ag1bv = ag1b_out.rearrange("(j r) t -> j r t", r=KVL + DR)

    # ==================== phase A: local latents ====================
    with tc.tile_pool(name="pA", bufs=1) as pa, \
         tc.tile_pool(name="pAw", bufs=3) as paw, \
         tc.tile_pool(name="pAt", bufs=2) as pat, \
         tc.tile_pool(name="psA", bufs=2, space="PSUM") as psa:

        x0 = pa.tile([128, 16, TSH], FR)
        for q4 in range(4):
            dma(x0[:, 4 * q4:4 * (q4 + 1), :],
                kp(hidT)[:, 4 * q4:4 * (q4 + 1), :].bitcast(FR))
        x0b = pa.tile([128, 16, TSH], BF)
        for k in range(16):
            nc.scalar.copy(x0b[:, k, :], x0[:, k, :])

        ss_ps = psa.tile([1, TSH], FP, tag="st")
        for k in range(16):
            sq = pat.tile([128, TSH], FR, tag="sq")
            nc.scalar.square(sq[:], x0[:, k, :])
            nc.tensor.matmul(ss_ps[:], fr(ones[:]), fr(sq[:]),
                             start=(k == 0), stop=(k == 15))
        rstd = pa.tile([1, TSH], FP)
        nc.scalar.activation(rstd[:], ss_ps[:], AF.Sqrt,
                             bias=epsb[0:1, :], scale=1.0 / H)
        nc.vector.reciprocal(rstd[:], rstd[:])
        bloc = pa.tile([128, TSH], FP)
        nc.gpsimd.partition_broadcast(bloc[:], rstd[:1, :])

        # kv latents first: their (small) gather flies during q_a compute,
        # and the q gather flies during the attention kv-path
        ckv_s = pa.tile([128, 4, TSH], FP)
        kr_raw = pa.tile([64, TSH], FP)
        for m in range(5):
            mc = 128 if m < 4 else 64
            wv = paw.tile([128, 16, 128], BF, tag="wa")
            dma(wv[:, :, :mc], kp(kva_wT[:, 128 * m:128 * m + mc]))
            ps = psa.tile([128, TSH], FP, tag="mm")
            for k in range(16):
                nc.tensor.matmul(ps[:mc, :], wv[:, k, :mc],
                                 x0b[:, k, :],
                                 start=(k == 0), stop=(k == 15))
            if m < 4:
                nc.vector.tensor_mul(ckv_s[:, m, :], ps[:], bloc[:])
            else:
                nc.vector.tensor_mul(kr_raw[:], ps[:64, :], bloc[:64, :])

        ss3 = psa.tile([1, TSH], FP, tag="st")
        for m in range(4):
            sq = pat.tile([128, TSH], FR, tag="sq")
            nc.scalar.square(sq[:], ckv_s[:, m, :])
            nc.tensor.matmul(ss3[:], fr(ones[:]), fr(sq[:]),
                             start=(m == 0), stop=(m == 3))
        r3 = pa.tile([1, TSH], FP)
        nc.scalar.activation(r3[:], ss3[:], AF.Sqrt,
                             bias=epsb[0:1, :], scale=1.0 / KVL)
        nc.vector.reciprocal(r3[:], r3[:])
        b3loc = pa.tile([128, TSH], FP)
        nc.gpsimd.partition_broadcast(b3loc[:], r3[:1, :])
        ckv_b = pa.tile([128, 4, TSH], BF)
        for m in range(4):
            nc.vector.tensor_mul(ckv_b[:, m, :], ckv_s[:, m, :], b3loc[:])

        # local k rope (rot rows pre-permuted to [A(32) B(32)] on host)
        cck = pa.tile([64, TSH], FP)
        ssk = pa.tile([64, TSH], FP)
        dma(cck[:], cc_k[:])
        dma(ssk[:], ss_k[:])
        kr_sh = pa.tile([64, TSH], FP)
        dma(kr_sh[0:32, :], kr_raw[32:64, :])
        dma(kr_sh[32:64, :], kr_raw[0:32, :])
        nc.vector.tensor_mul(kr_sh[:], kr_sh[:], ssk[:])
        kr = pa.tile([64, TSH], FP)
        nc.vector.tensor_mul(kr[:], kr_raw[:], cck[:])
        kr_b = pa.tile([64, TSH], BF)
        nc.vector.tensor_add(kr_b[:], kr[:], kr_sh[:])

        dma(ag1b_in[0:KVL, :].rearrange("(m p) t -> p m t", p=128),
            ckv_b[:])
        dma(ag1b_in[KVL:KVL + DR, :], kr_b[:])
        nc.gpsimd.collective_compute(
            "AllGather", ALU.bypass, replica_groups=RG,
            ins=[ag1b_in.opt()], outs=[ag1b_out.opt()])

        qa_s = pa.tile([128, 12, TSH], FP)
        for m in range(12):
            wa = paw.tile([128, 16, 128], BF, tag="wa")
            dma(wa[:], kp(qa_wT[:, 128 * m:128 * (m + 1)]))
            ps = psa.tile([128, TSH], FP, tag="mm")
            for k in range(16):
                nc.tensor.matmul(ps[:], wa[:, k, :], x0b[:, k, :],
                                 start=(k == 0), stop=(k == 15))
            nc.vector.tensor_mul(qa_s[:, m, :], ps[:], bloc[:])

        ss2 = psa.tile([1, TSH], FP, tag="st")
        for m in range(12):
            sq = pat.tile([128, TSH], FR, tag="sq")
            nc.scalar.square(sq[:], qa_s[:, m, :])
            nc.tensor.matmul(ss2[:], fr(ones[:]), fr(sq[:]),
                             start=(m == 0), stop=(m == 11))
        r2 = pa.tile([1, TSH], FP)
        nc.scalar.activation(r2[:], ss2[:], AF.Sqrt,
                             bias=epsb[0:1, :], scale=1.0 / QL)
        nc.vector.reciprocal(r2[:], r2[:])
        b2loc = pa.tile([128, TSH], FP)
        nc.gpsimd.partition_broadcast(b2loc[:], r2[:1, :])
        qa_b = pa.tile([128, 12, TSH], BF)
        for m in range(12):
            nc.vector.tensor_mul(qa_b[:, m, :], qa_s[:, m, :], b2loc[:])
        dma(ag1a_in[:, :].rearrange("(m p) t -> p m t", p=128), qa_b[:])

    nc.gpsimd.collective_compute(
        "AllGather", ALU.bypass, replica_groups=RG,
        ins=[ag1a_in.opt()], outs=[ag1a_out.opt()])

    # ==================== attention ====================
    with tc.tile_pool(name="att", bufs=1) as at, \
         tc.tile_pool(name="atp", bufs=2) as atp, \
         tc.tile_pool(name="psT", bufs=2, space="PSUM") as pst:

        qn = at.tile([128, 2, T], FR)
        qr = at.tile([128, T], FR)
        qr1 = at.tile([64, T], FR)
        kn = at.tile([128, 2, T], FR)
        krotg = at.tile([64, T], FR)
        vt = at.tile([128, 16, TSH], FR)
        attn = at.tile([128, 2, T], BF)
        maskT = at.tile([128, 4, 512], BF)
        dma(maskT[:], kp(maskT_d))

        with tc.tile_pool(name="proj", bufs=1) as pj, \
             tc.tile_pool(name="projs", bufs=2) as pjs:

            qb_sb = pj.tile([128, 12, HPC * DQK], BF)
            dma(qb_sb[:], kp(qb_wT))
            kvb_sb = pj.tile([128, 4, HPC * 256], BF)
            dma(kvb_sb[:], kp(kvb_wT))

            krotg_b = pj.tile([64, T], BF)
            nc.scalar.dma_start(
                krotg_b[:].rearrange("p (j t) -> p j t", t=TSH),
                ag1bv[:, KVL:KVL + DR, :].rearrange("j p t -> p j t"))
            nc.scalar.copy(krotg[:], krotg_b[:])

            for n in range(NCORE):          # kv path, needs only ag1b
                nsl = slice(TSH * n, TSH * (n + 1))
                lat_n = pjs.tile([128, 4, TSH], BF, tag="latn")
                nc.scalar.dma_start(
                    lat_n[:], ag1bv[n, 0:KVL, :].rearrange(
                        "(k p) t -> p k t", p=128))
                for h in range(2):
                    ps = pst.tile([128, TSH], FP, tag="mm")
                    for k in range(4):
                        nc.tensor.matmul(
                            ps[:], kvb_sb[:, k, 128 * h:128 * (h + 1)],
                            lat_n[:, k, :],
                            start=(k == 0), stop=(k == 3))
                    nc.scalar.copy(kn[:, h, nsl], ps[:])
                for s2 in range(2):
                    ps = pst.tile([128, TSH], FP, tag="mm")
                    for k in range(4):
                        nc.tensor.matmul(
                            ps[:], lat_n[:, k, 128 * s2:128 * (s2 + 1)],
                            kvb_sb[:, k, 256:512],
                            start=(k == 0), stop=(k == 3))
                    sp = 2 * n + s2
                    nc.scalar.copy(vt[:, sp, :], ps[:])

            for n in range(NCORE):          # q path, needs ag1a
                nsl = slice(TSH * n, TSH * (n + 1))
                qrhs = pjs.tile([128, 12, TSH], BF, tag="qrhs")
                nc.scalar.dma_start(
                    qrhs[:], ag1av[n].rearrange("(k p) t -> p k t", p=128))
                qro = pjs.tile([128, TSH], FP, tag="qro")
                for m in range(3):
                    ps = pst.tile([128, TSH], FP, tag="mm")
                    for k in range(12):
                        nc.tensor.matmul(
                            ps[:], qb_sb[:, k, 128 * m:128 * (m + 1)],
                            qrhs[:, k, :],
                            start=(k == 0), stop=(k == 11))
                    dst = qn[:, m, nsl] if m < 2 else qro[:]
                    nc.scalar.copy(dst, ps[:])
                # rope this token tile
                qsh = pjs.tile([128, TSH], FP, tag="qsh")
                dma(qsh[0:32, :], qro[32:64, :])
                dma(qsh[32:64, :], qro[0:32, :])
                dma(qsh[64:96, :], qro[96:128, :])
                dma(qsh[96:128, :], qro[64:96, :])
                ccn = pjs.tile([128, TSH], FP, tag="ccn")
                dma(ccn[:], cc_q[:, nsl])
                ssn = pjs.tile([128, TSH], FP, tag="ssn")
                dma(ssn[:], ss_q[:, nsl])
                nc.vector.tensor_mul(qsh[:], qsh[:], ssn[:])
                nc.vector.tensor_mul(qr[:, nsl], qro[:], ccn[:])
                nc.vector.tensor_add(qr[:, nsl], qr[:, nsl], qsh[:])

        dma(qr1[:], qr[64:128, :])

        # flash attention, scores transposed [s'_p, s_f]
        for b_ in range(2):
            for h in range(2):
                for sqi in range(2):
                    q0 = 1024 * b_ + 512 * sqi
                    qsl = slice(q0, q0 + 512)
                    nk = 4 * (sqi + 1)
                    aps = pst.tile([128, 512], FP, tag="av")
                    dps = pst.tile([1, 512], FP, tag="dn")
                    for sk in range(nk):
                        k0 = 1024 * b_ + 128 * sk
                        ksl = slice(k0, k0 + 128)
                        sps = pst.tile([128, 512], FP, tag="sc")
                        nc.tensor.matmul(sps[:], fr(kn[:, h, ksl]),
                                         fr(qn[:, h, qsl]),
                                         start=True, stop=False)
                        qrh = qr[0:64, qsl] if h == 0 else qr1[:, qsl]
                        nc.tensor.matmul(
                            sps[:], fr(krotg[:, ksl]), fr(qrh),
                            start=False, stop=True)
                        pr = atp.tile([128, 512], FR, tag="pr", bufs=2)
                        nc.scalar.activation(pr[:], sps[:], AF.Exp,
                                             scale=SCALING)
                        if sk >= 4 * sqi:
                            nc.vector.tensor_mul(
                                pr[:], pr[:], maskT[:, sk - 4 * sqi, :])
                        nc.tensor.matmul(
                            aps[:], fr(vt[:, 8 * b_ + sk,
                                          128 * h:128 * (h + 1)]),
                            fr(pr[:]), start=(sk == 0),
                            stop=(sk == nk - 1), skip_group_check=True)
                        nc.tensor.matmul(
                            dps[:], fr(ones[:]), fr(pr[:]),
                            start=(sk == 0), stop=(sk == nk - 1),
                            skip_group_check=True)
                    rd = atp.tile([1, 512], FP, tag="rd", bufs=1)
                    nc.vector.reciprocal(rd[:], dps[:])
                    rdb = atp.tile([128, 512], FP, tag="rdb", bufs=1)
                    nc.gpsimd.partition_broadcast(rdb[:], rd[:1, :])
                    nc.vector.tensor_mul(attn[:, h, qsl], aps[:], rdb[:])

        a2av = a2a_in.rearrange("(j h p) t -> j p h t", h=2, p=128)
        for j in range(NCORE):
            dma(a2av[j], attn[:, :, TSH * j:TSH * (j + 1)])

    nc.gpsimd.collective_compute(
        "AllToAll", ALU.bypass, replica_groups=RG,
        ins=[a2a_in.opt()], outs=[a2a_out.opt()])

    # ==================== o_proj + ln2 + router ====================
    late_cm = tc.tile_pool(name="late", bufs=1)
    late = late_cm.__enter__()
    x2s = late.tile([128, 16, TSH], FP)
    hs = late.tile([128, 16, TSH], FR)
    hb = late.tile([128, 16, TSH], BF)
    bce = late.tile([128, T], FP)
    # routed-expert gate/up weights (loaded during o_proj, below)
    gw_sb = late.tile([128, 16, INTER], BF)
    uw_sb = late.tile([128, 16, INTER], BF)

    with tc.tile_pool(name="op", bufs=1) as po, \
         tc.tile_pool(name="opw2", bufs=2) as pow2, \
         tc.tile_pool(name="opt", bufs=2) as pot, \
         tc.tile_pool(name="psO", bufs=2, space="PSUM") as pso:

        x0r = po.tile([128, 16, TSH], FP)
        dma(x0r[:], kp(hidT))
        attn_sb = po.tile([128, 16, TSH], BF)
        nc.scalar.dma_start(attn_sb[:], kp(a2a_out[:, :]))

        for m in range(16):
            ow = po.tile([128, 16, 128], BF, tag="ow", bufs=6)
            dma(ow[:], kp(o_wT[:, 128 * m:128 * (m + 1)]))
            ps = pso.tile([128, TSH], FP, tag="mm")
            for k in range(16):
                nc.tensor.matmul(ps[:], ow[:, k, :],
                                 attn_sb[:, k, :],
                                 start=(k == 0), stop=(k == 15))
            nc.vector.tensor_add(x2s[:, m, :], ps[:], x0r[:, m, :])

        dma(gw_sb[:], kp(g_wT))
        dma(uw_sb[:], kp(u_wT))

        ss4 = pso.tile([1, TSH], FP, tag="st", bufs=1)
        for k in range(16):
            sq = pot.tile([128, TSH], FR, tag="sq")
            nc.scalar.square(sq[:], x2s[:, k, :])
            nc.tensor.matmul(ss4[:], fr(ones[:]), fr(sq[:]),
                             start=(k == 0), stop=(k == 15))
        r4 = po.tile([1, TSH], FP)
        nc.scalar.activation(r4[:], ss4[:], AF.Sqrt,
                             bias=epsb[0:1, :], scale=1.0 / H)
        nc.vector.reciprocal(r4[:], r4[:])
        b4 = po.tile([128, TSH], FP)
        nc.gpsimd.partition_broadcast(b4[:], r4[:1, :])
        for m in range(16):
            nc.vector.tensor_mul(hs[:, m, :], x2s[:, m, :], b4[:])
            nc.scalar.copy(hb[:, m, :], hs[:, m, :])
        dma(ag2_in[:, :].rearrange("(m p) t -> p m t", p=128), hb[:])
        # gather h now; router + shared expert compute while it flies
        nc.gpsimd.collective_compute(
            "AllGather", ALU.bypass, replica_groups=RG,
            ins=[ag2_in.opt()], outs=[ag2_out.opt()])

        # shared expert (needs only local h) — overlaps the gathers
        act2 = po.tile([128, 6, TSH], BF)
        for m in range(6):
            sgw = pow2.tile([128, 16, 128], BF, tag="sgw", bufs=2)
            dma(sgw[:], kp(sg_wT[:, 128 * m:128 * (m + 1)]))
            g2 = pso.tile([128, 512], FP, tag="smg")
            for k in range(16):
                nc.tensor.matmul(g2[:, 0:TSH], sgw[:, k, :], hb[:, k, :],
                                 start=(k == 0), stop=(k == 15))
            g2s = pot.tile([128, TSH], FP, tag="g2s")
            nc.scalar.activation(g2s[:], g2[:, 0:TSH], AF.Sigmoid)
            nc.vector.tensor_mul(g2s[:], g2[:, 0:TSH], g2s[:])
            suw = pow2.tile([128, 16, 128], BF, tag="suw", bufs=2)
            dma(suw[:], kp(su_wT[:, 128 * m:128 * (m + 1)]))
            u2 = pso.tile([128, 512], FP, tag="smg")
            for k in range(16):
                nc.tensor.matmul(u2[:, 0:TSH], suw[:, k, :], hb[:, k, :],
                                 start=(k == 0), stop=(k == 15))
            nc.vector.tensor_mul(act2[:, m, :], u2[:, 0:TSH], g2s[:])
        for m in range(16):
            sdw = pow2.tile([128, 6, 128], BF, tag="sdw")
            dma(sdw[:], kp(sd_wT[:, 128 * m:128 * (m + 1)]))
            d2 = pso.tile([128, 512], FP, tag="smd", bufs=1)
            for k in range(6):
                nc.tensor.matmul(d2[:, 0:TSH], sdw[:, k, :],
                                 act2[:, k, :],
                                 start=(k == 0), stop=(k == 5))
            nc.vector.tensor_add(x2s[:, m, :], d2[:, 0:TSH], x2s[:, m, :])

        # router (fp32 matmuls)
        rw_sb = po.tile([128, 16, NE], FP)
        dma(rw_sb[:], kp(r_wT))
        rb_sb = po.tile([NE, 1], FP)
        dma(rb_sb[:], r_bias[:])
        Gm_s = po.tile([NE, NG], FP)
        dma(Gm_s[:], Gm_d[:])
        Dg_s = po.tile([NG, 16], FP)
        dma(Dg_s[:], Dg_d[:])
        Rg_s = po.tile([16, NG], FP)
        dma(Rg_s[:], Rg_d[:])
        Em_s = po.tile([NG, NE], FP)
        dma(Em_s[:], Em_d[:])
        De_s = po.tile([NE, 64], FP)
        dma(De_s[:], De_d[:])
        Re_s = po.tile([64, NE], FP)
        dma(Re_s[:], Re_d[:])

        lg = pso.tile([NE, TSH], FP, tag="rt", bufs=1)
        for k in range(16):
            nc.tensor.matmul(lg[:], rw_sb[:, k, :], hs[:, k, :].bitcast(FP),
                             start=(k == 0), stop=(k == 15))
        sr = po.tile([NE, TSH], FP)
        nc.scalar.activation(sr[:], lg[:], AF.Sigmoid)
        sc_t = po.tile([NE, TSH], FP)
        nc.vector.tensor_scalar(sc_t[:], sr[:], rb_sb[:, 0:1], None, ALU.add)
        gs_ps = pso.tile([NG, TSH], FP, tag="rt", bufs=1)
        nc.tensor.matmul(gs_ps[:], Gm_s[:], sc_t[:])
        gs_sb = po.tile([NG, TSH], FP)
        nc.scalar.copy(gs_sb[:], gs_ps[:])
        gd_ps = pso.tile([16, TSH], FP, tag="rt", bufs=1)
        nc.tensor.matmul(gd_ps[:], Dg_s[:], gs_sb[:])
        gp = po.tile([16, TSH], FP)
        nc.vector.tensor_scalar(gp[:], gd_ps[:], 0.0, None, ALU.is_gt)
        gc_ps = pso.tile([NG, TSH], FP, tag="rt", bufs=1)
        nc.tensor.matmul(gc_ps[:], Rg_s[:], gp[:])
        gm = po.tile([NG, TSH], FP)
        nc.vector.tensor_scalar(gm[:], gc_ps[:], 2.0, None, ALU.is_lt)
        em_ps = pso.tile([NE, TSH], FP, tag="rt", bufs=1)
        nc.tensor.matmul(em_ps[:], Em_s[:], gm[:])
        msk = po.tile([NE, TSH], FP)
        nc.vector.tensor_mul(msk[:], em_ps[:], sc_t[:])
        ed_ps = pso.tile([64, TSH], FP, tag="rt", bufs=1)
        nc.tensor.matmul(ed_ps[:], De_s[:], msk[:])
        ep = po.tile([64, TSH], FP)
        nc.vector.tensor_scalar(ep[:], ed_ps[:], 0.0, None, ALU.is_gt)
        ec_ps = pso.tile([NE, TSH], FP, tag="rt", bufs=1)
        nc.tensor.matmul(ec_ps[:], Re_s[:], ep[:])
        es = po.tile([NE, TSH], FP)
        nc.vector.tensor_scalar(es[:], ec_ps[:], 2.0, None, ALU.is_lt)
        w_sb = po.tile([NE, TSH], FP)
        nc.vector.tensor_mul(w_sb[:], es[:], sr[:])
        ws_ps = pso.tile([1, TSH], FP, tag="rt", bufs=1)
        nc.tensor.matmul(ws_ps[:], ones[0:NE, :], w_sb[:])
        wse = po.tile([1, TSH], FP)
        nc.vector.tensor_scalar(wse[:], ws_ps[:], 1e-20, None, ALU.add)
        nc.vector.reciprocal(wse[:], wse[:])
        wb = po.tile([NE, TSH], FP)
        nc.gpsimd.partition_broadcast(wb[:], wse[:1, :])
        dw_sb = po.tile([NE, TSH], FP)
        nc.vector.scalar_tensor_tensor(dw_sb[:], w_sb[:], RSF, wb[:],
                                       ALU.mult, ALU.mult)
        dwb = po.tile([NE, TSH], BF)
        nc.scalar.copy(dwb[:], dw_sb[:])
        dma(agw_in[:, :], dwb[:])
        nc.gpsimd.collective_compute(
            "AllGather", ALU.bypass, replica_groups=RG,
            ins=[agw_in.opt()], outs=[agw_out.opt()])

        oh_sb = po.tile([NE, 1], BF)
        dma(oh_sb[:], onehot[:])
        dwg = po.tile([NE, NCORE, TSH], BF)
        nc.scalar.dma_start(
            dwg[:], agw_out[:, :].rearrange("(j p) t -> p j t", p=NE))
        for jj in range(4):
            ewp = pso.tile([1, 512], FP, tag="rt", bufs=1)
            for q in range(2):
                nc.tensor.matmul(ewp[:, TSH * q:TSH * (q + 1)],
                                 oh_sb[:], dwg[:, 2 * jj + q, :])
            nc.scalar.copy(bce[0:1, 512 * jj:512 * (jj + 1)], ewp[:])
        nc.gpsimd.partition_broadcast(bce[:], bce[0:1, :])

    # ===== MoE: expert-parallel (1 expert/core, resident weights, bf16) =====
    ag2v = ag2_out.rearrange("(j r) t -> j r t", r=H)
    with tc.tile_pool(name="moe", bufs=1) as pm, \
         tc.tile_pool(name="moet", bufs=2) as pmt, \
         tc.tile_pool(name="moew", bufs=3) as pmw, \
         tc.tile_pool(name="psM", bufs=2, space="PSUM") as psm:


        # phase 1: gate/up for all 4 token blocks, acts kept resident
        acts = pm.tile([128, 6, 4, 512], BF)
        for n in range(4):
            hb_n = pmt.tile([128, 16, 2, TSH], BF, tag="hb", bufs=2)
            for jj in range(2):
                nc.scalar.dma_start(
                    hb_n[:, :, jj, :],
                    ag2v[2 * n + jj].rearrange("(k p) t -> p k t", p=128))
            for m in range(6):
                gp_ = psm.tile([128, 512], FP, tag="mg")
                for k in range(16):
                    nc.tensor.matmul(gp_[:],
                                     gw_sb[:, k, 128 * m:128 * (m + 1)],
                                     hb_n[:, k, :, :],
                                     start=(k == 0), stop=(k == 15))
                gsi = pmt.tile([128, 512], FP, tag="gsi")
                nc.scalar.activation(gsi[:], gp_[:], AF.Sigmoid)
                nc.vector.tensor_mul(gsi[:], gp_[:], gsi[:])
                up_ = psm.tile([128, 512], FP, tag="mg")
                for k in range(16):
                    nc.tensor.matmul(up_[:],
                                     uw_sb[:, k, 128 * m:128 * (m + 1)],
                                     hb_n[:, k, :, :],
                                     start=(k == 0), stop=(k == 15))
                nc.vector.tensor_mul(acts[:, m, n, :], up_[:], gsi[:])

        # phase 2: down-proj in 4 m-chunks; each chunk's ReduceScatter
        # overlaps the next chunk's matmuls
        for c in range(4):
            rsv = rs_ins[c].rearrange("(j mi p) t -> j mi p t", j=NCORE,
                                      mi=4)
            for mi in range(4):
                m = 4 * c + mi
                dwn = pmw.tile([128, 6, 128], BF, tag="dwm", bufs=3)
                dma(dwn[:], kp(d_wT[:, 128 * m:128 * (m + 1)]))
                eo = pmw.tile([128, 4, 512], BF, tag="eo", bufs=2)
                for n in range(4):
                    nsl = slice(512 * n, 512 * (n + 1))
                    dp = psm.tile([128, 512], FP, tag="md", bufs=3)
                    for k in range(6):
                        nc.tensor.matmul(dp[:],
                                         dwn[:, k, :],
                                         acts[:, k, n, :],
                                         start=(k == 0), stop=(k == 5))
                    last_eo = nc.vector.tensor_mul(eo[:, n, :], dp[:],
                                                   bce[:, nsl])
                # one store per mi; SBUF side stays partition-outermost
                # ((n j) merge is order-preserving), DRAM side reordered
                eng = nc.scalar if mi % 2 == 0 else nc.sync
                eng.dma_start(
                    rsv[:, mi].rearrange("j p t -> p j t"),
                    eo[:].rearrange("p n (j t) -> p (n j) t", j=2))
            nc.gpsimd.collective_compute(
                "ReduceScatter", ALU.add, replica_groups=RG,
                ins=[rs_ins[c].opt()], outs=[rs_outs[c].opt()])

        # final adds last so no RS-gated op ever sits ahead of compute in
        # the strict-FIFO DVE queue
        for c in range(4):
            for mi in range(4):
                m = 4 * c + mi
                rsb = pmt.tile([128, TSH], BF, tag="rsb")
                nc.gpsimd.dma_start(rsb[:], kp(rs_outs[c])[:, mi, :])
                fin = pmt.tile([128, TSH], FP, tag="fin")
                fa = nc.vector.tensor_add(fin[:], rsb[:], x2s[:, m, :])
                # keep RS-gated adds behind every eo mul in the DVE FIFO
                add_dep_helper(fa.ins, last_eo.ins, sync=False,
                               reason="fin after all eo muls")
                nc.gpsimd.dma_start(out[128 * m:128 * (m + 1), :], fin[:])

    late_cm.__exit__(None, None, None)
    pp_cm.__exit__(None, None, None)
    dram_cm.__exit__(None, None, None)
    tcx.__exit__(None, None, None)

    nc.compile()
    return nc


# --------------------------------------------------------------------------
# host side
# --------------------------------------------------------------------------

_PERM64 = np.concatenate([np.arange(0, 64, 2), np.arange(1, 64, 2)])


def _routing_mats():
    Gm = np.zeros((NE, NG), np.float32)
    for g in range(NG):
        Gm[2 * g, g] = 1.0
        Gm[2 * g + 1, g] = 1.0
    Dg = np.zeros((NG, NG * NG), np.float32)
    Rg = np.zeros((NG * NG, NG), np.float32)
    for i in range(NG):
        for j in range(NG):
            p = i * NG + j
            Dg[i, p] += 1.0
            Dg[j, p] -= 1.0
            Rg[p, j] = 1.0
    Em = np.zeros((NG, NE), np.float32)
    for g in range(NG):
        Em[g, 2 * g] = 1.0
        Em[g, 2 * g + 1] = 1.0
    De = np.zeros((NE, NE * NE), np.float32)
    Re = np.zeros((NE * NE, NE), np.float32)
    for i in range(NE):
        for j in range(NE):
            p = i * NE + j
            De[i, p] += 1.0
            De[j, p] -= 1.0
            Re[p, j] = 1.0
    return Gm, Dg, Rg, Em, De, Re


def _c(a):
    return np.ascontiguousarray(a, dtype=np.float32)


def _bfc(a):
    return np.ascontiguousarray(np.asarray(a, np.float32).astype(
        ml_dtypes.bfloat16))


def make_in_maps(inputs):
    f32 = np.float32
    hs_ = np.asarray(inputs["hidden_states"], f32).reshape(T, H)
    cos = np.asarray(inputs["cos"], f32).reshape(T, DR)
    sin = np.asarray(inputs["sin"], f32).reshape(T, DR)
    ln1 = np.asarray(inputs["ln1_w"], f32)
    ln2 = np.asarray(inputs["ln2_w"], f32)
    qaln = np.asarray(inputs["q_a_ln_w"], f32)
    kvln = np.asarray(inputs["kv_a_ln_w"], f32)

    qa_w = np.asarray(inputs["q_a_w"], f32) * ln1[None, :]
    kva_w = np.asarray(inputs["kv_a_w"], f32) * ln1[None, :]
    kva_w = np.concatenate([kva_w[:KVL], kva_w[KVL:][_PERM64]], 0)
    qb_w = np.asarray(inputs["q_b_w"], f32) * qaln[None, :]
    kvb_w = np.asarray(inputs["kv_b_w"], f32) * kvln[None, :]
    o_w = np.asarray(inputs["o_w"], f32)
    r_w = np.asarray(inputs["router_w"], f32) * ln2[None, :]
    r_b = np.asarray(inputs["router_bias"], f32)
    g_w = np.asarray(inputs["gate_w"], f32) * ln2[None, None, :]
    u_w = np.asarray(inputs["up_w"], f32) * ln2[None, None, :]
    d_w = np.asarray(inputs["down_w"], f32)
    sg_w = np.asarray(inputs["sh_gate_w"], f32) * ln2[None, :]
    su_w = np.asarray(inputs["sh_up_w"], f32) * ln2[None, :]
    sd_w = np.asarray(inputs["sh_down_w"], f32)

    cosT = cos.T
    sinT = sin.T
    cc_q = np.concatenate([cosT[0:32], cosT[32:64]] * 2, 0)
    ss_q = np.concatenate([-sinT[0:32], sinT[32:64]] * 2, 0)
    maskT = np.triu(np.ones((512, 512), np.float32))
    Gm, Dg, Rg, Em, De, Re = _routing_mats()

    shared = dict(
        qa_wT=_bfc(qa_w.T), kva_wT=_bfc(kva_w.T), o_wT=_bfc(o_w.T),
        r_wT=_c(r_w.T), r_bias=_c(r_b.reshape(NE, 1)),
        sg_wT=_bfc(sg_w.T), su_wT=_bfc(su_w.T), sd_wT=_bfc(sd_w.T),
        cc_q=_c(cc_q), ss_q=_c(ss_q), maskT=_bfc(maskT),
        Gm=_c(Gm), Dg=_c(Dg), Rg=_c(Rg), Em=_c(Em), De=_c(De), Re=_c(Re),
    )

    in_maps = []
    for c in range(NCORE):
        tsl = slice(TSH * c, TSH * (c + 1))
        h0, h1 = 2 * c, 2 * c + 1
        qb_cols = [qb_w[DQK * h0:DQK * h0 + DN],
                   qb_w[DQK * h1:DQK * h1 + DN]]
        for h in (h0, h1):
            rot = qb_w[DQK * h + DN:DQK * (h + 1)]
            qb_cols.append(rot[0::2])
            qb_cols.append(rot[1::2])
        qb_c = np.concatenate(qb_cols, 0)              # [384, QL]
        kvb_c = np.concatenate(
            [kvb_w[256 * h0:256 * h0 + 128],
             kvb_w[256 * h1:256 * h1 + 128],
             kvb_w[256 * h0 + 128:256 * h0 + 256],
             kvb_w[256 * h1 + 128:256 * h1 + 256]], 0)  # [512, KVL]
        oh = np.zeros((NE, 1), np.float32)
        oh[c, 0] = 1.0
        m = dict(shared)
        m.update(
            hidT=_c(hs_[tsl].T),
            qb_wT=_bfc(qb_c.T), kvb_wT=_bfc(kvb_c.T),
            cc_k=_c(cosT[:, tsl]),
            ss_k=_c(np.concatenate([-sinT[0:32, tsl],
                                    sinT[32:64, tsl]], 0)),
            onehot=_bfc(oh),
            g_wT=_bfc(g_w[c].T), u_wT=_bfc(u_w[c].T), d_wT=_bfc(d_w[c].T),
        )
        in_maps.append(m)
    return in_maps


_NC_CACHE = None


def _get_nc():
    global _NC_CACHE
    if _NC_CACHE is None:
        _NC_CACHE = build_program()
    return _NC_CACHE


def kernel(**inputs) -> np.ndarray:
    nc = _get_nc()
    in_maps = make_in_maps(inputs)
    res = bass_utils.run_bass_kernel_spmd(nc, in_maps,
                                          core_ids=list(range(NCORE)))
    full = np.empty((H, T), np.float32)
    for c in range(NCORE):
        full[:, TSH * c:TSH * (c + 1)] = res.results[c]["out"]
    return np.ascontiguousarray(full.T).reshape(B, S, H)

